# revision 47
# baseline (speedup 1.0000x reference)
"""Trainium2 Bass kernel for CoarseMatching (mutual-nearest-neighbor + border/thr masking).

Contract: kernel(**inputs) takes the FULL inputs (conf_matrix [4,4800,4800] f32 plus
scalar grid dims) and returns the FULL outputs (mconf [4,4800] f32, mask_v [4,4800] bool,
all_j_ids [4,4800] int32), matching reference() exactly.

Strategy (8 NeuronCores, single kernel launch, ~213 us HW):
  - Shard each of the 4 samples' rows across 2 cores -> per-core slab [2400, 4800].
  - One streaming pass per core over 19 row-tiles [128, 4800]; per tile the DVE does
    exactly two full passes over the data (the floor for exact mutual-NN: every
    combining op - tensor_reduce, fp32 tensor_tensor, tensor_scalar+accum - runs at
    1 elem/cycle/lane on TRN2, and no other engine can legally run 2-tensor max in
    this toolchain):
      1. chunked row maxima: reduce_max over [128, 25, 192] -> chunk maxima
      2. running column-max accumulator (tensor_max)
    Tile 0 streams in as five 5-chunk pieces (and tile 1 as two halves) so the DVE
    starts ~2 us after the fixed preamble and is fed gaplessly; tile 0's colacc
    contribution uses 2x-mode tensor_copy. The last tile's colacc updates are
    split into four 128-aligned column groups so the PE-transpose epilogue starts
    while the DVE still drains.
  - The full chunk-max array (ch_out, 243 KB) is DMA'd out once the last chunk
    pass lands; the host derives rowmax / winning chunk / tie multiplicity from
    it directly (cheaper than the old on-device A-encoding tail math). The
    PE-transpose + DVE-reduce epilogue turns the column accumulator into
    per-core partial colmax (cmx).
  - Host: combine partial colmaxes per sample pair; re-read only the 192-wide
    window of the raw input per row to find the first mask index (conf==rowmax
    AND conf==colmax AND border/thr). Rows with multi-chunk row-max ties (~5
    per run) are recomputed exactly from the raw row. Bitwise-exact vs the
    reference.
"""

import sys

if "/opt/trn_rl_repo" not in sys.path:
    sys.path.insert(0, "/opt/trn_rl_repo")

import numpy as np

import concourse.bass as bass
import concourse.mybir as mybir
from concourse.masks import make_identity
from concourse.tile import TileContext
from concourse.vector_clock import ScopedClock, VectorClock
from concourse.bass_utils import run_bass_kernel_spmd

THR = 0.2
BORDER_RM = 2

N = 4
L = 4800
S = 4800
R = L // 2          # rows per core
P = 128
NFULL = R // P      # 18 full tiles
TAIL = R - NFULL * P  # 96
NT = NFULL + 1
NCHUNK = (S + P - 1) // P  # 38 column chunks for colmax transpose-reduce

CW = 192            # row-chunk width for rowmax/argmax chunking
NC_ = S // CW       # 25 chunks per row
WBASE = 65536.0     # chunk-id encoding base (exact in f32 up to 2^24 sums)

_BUILT = None  # cached (nc,) bass program


def _patched_drain_and_barrier(self, tick_clock, wait_clock):
    # The stock tile-exit drain carries one sem-wait per live semaphore; this
    # walrus build only encodes 1 sync wait per CTRL instruction. Split the
    # waits across single-wait SP NOPs, then drain with none attached.
    gc = tick_clock.global_clock
    vc = gc[None] if hasattr(gc, "items") else gc
    n = len(vc)
    for p in range(n):
        if vc[p] > 0:
            sub = [0] * n
            sub[p] = vc[p]
            nop_inst = self.nc.sync.nop()
            wait_clock.add_sem_waits(nop_inst.ins, ScopedClock({None: VectorClock(sub)}))
    self.nc.sync.drain()
    self.nc.all_engine_barrier()
    assert self.sems is not None
    popped = self.nc._tile_sem_poison_stack.pop()
    assert popped is self._sem_poison
    self.nc.clear_and_free_semaphores(list(self.sems.allocated().values()))
    self.nc.all_engine_barrier()


def _legalize_waits(nc):
    """This walrus build encodes at most ONE sync wait per instruction; Tile's
    scheduler attaches up to 4. Split the extras onto same-engine NOPs placed
    immediately before the instruction (same program order, same semantics)."""
    ctr = [0]

    def mknop(engine, wait):
        ctr[0] += 1
        return mybir.InstNoOp(
            name=f"I-wsplit-{ctr[0]}",
            engine=engine,
            ins=[],
            outs=[],
            sync_info=mybir.SyncInfo(on_wait=[wait], on_update=[]),
        )

    f = nc.m.functions[0]
    for bb in f.blocks:
        insts = list(bb.instructions)
        out = []
        changed = False
        for inst in insts:
            si = inst.sync_info
            waits = list(si.on_wait) if si is not None else []
            if len(waits) > 1:
                ups = list(si.on_update) if si is not None else []
                for w in waits[:-1]:
                    out.append(mknop(inst.engine, w))
                inst.sync_info = mybir.SyncInfo(on_wait=[waits[-1]], on_update=ups)
                changed = True
            out.append(inst)
        if changed:
            bb.instructions = out


def _build():
    global _BUILT
    if _BUILT is not None:
        return _BUILT

    TileContext._drain_and_barrier = _patched_drain_and_barrier

    nc = bass.Bass("TRN2")
    f32 = mybir.dt.float32

    x = nc.dram_tensor("x", [R, S], f32, kind="ExternalInput")
    ch_out = nc.dram_tensor("ch_out", [P, NT * NC_], f32, kind="ExternalOutput")
    cmx = nc.dram_tensor("cmx", [P, NCHUNK], f32, kind="ExternalOutput")

    with TileContext(nc) as tc:
        with (
            tc.tile_pool(name="data", bufs=6) as dpool,
            tc.tile_pool(name="acc", bufs=1) as apool,
            tc.tile_pool(name="psum", bufs=2, space="PSUM") as ppool,
        ):
            colacc = apool.tile([P, S], f32)
            chunkall = apool.tile([P, NT * NC_], f32)
            cm_sb = apool.tile([P, NCHUNK], f32)
            ident = apool.tile([P, P], f32)

            # Column groups for the last tile's colacc update + epilogue
            # transposes: 128-aligned so each group's PE transposes can start
            # as soon as that group's final update lands.
            GB = [0, 1280, 2560, 3840, S]  # chunks 0-9, 10-19, 20-29, 30-37

            # Tile 0 first as five 5-chunk pieces so the DVE starts ~2 us
            # after the preamble and is fed continuously (piece transfer
            # ~1.5 us vs ~1.6 us of DVE work per piece); its colacc
            # contribution is a 2x-mode tensor_copy per piece. Tile 1 in two
            # halves to bridge into the steady stream; tiles 2..17 whole;
            # tile 18 (96 rows) last with per-group colacc updates feeding
            # the PE-transpose epilogue.
            # Piece sizes grow 3,4,5,6,7 chunks: the first transfer runs cold
            # (~130 GB/s), so a small first piece starts the DVE sooner and
            # later pieces stay arrival-paced.
            PB = [0, 3, 7, 12, 18, NC_]
            for k in range(5):
                lo, hi = PB[k] * CW, PB[k + 1] * CW
                tp = apool.tile([P, hi - lo], f32, tag=f"t0_{k}")
                nc.sync.dma_start(tp[:, :], x[:P, lo:hi])
                nc.vector.reduce_max(
                    out=chunkall[:, lo // CW:hi // CW],
                    in_=tp[:, :].rearrange("p (c w) -> p c w", w=CW),
                    axis=mybir.AxisListType.X,
                )
                nc.vector.tensor_copy(colacc[:, lo:hi], tp[:, :])
                if k == 0:
                    # Epilogue-only setup, placed in the DMA-wait gaps.
                    make_identity(nc, ident[:, :])
                    nc.vector.memset(chunkall[TAIL:, NFULL * NC_:], 0.0)

            # Tile 1 in two halves (12|13 chunks).
            mid1 = 12 * CW
            for lo, hi in ((0, mid1), (mid1, S)):
                th = apool.tile([P, hi - lo], f32, tag=f"t1_{lo}")
                nc.sync.dma_start(th[:, :], x[P:2 * P, lo:hi])
                nc.vector.reduce_max(
                    out=chunkall[:, NC_ + lo // CW:NC_ + hi // CW],
                    in_=th[:, :].rearrange("p (c w) -> p c w", w=CW),
                    axis=mybir.AxisListType.X,
                )
                nc.vector.tensor_max(
                    colacc[:, lo:hi], colacc[:, lo:hi], th[:, :]
                )

            for t in range(2, NT):
                p = P if t < NFULL else TAIL
                r0 = t * P
                tile = dpool.tile([P, S], f32, tag="tile")
                nc.sync.dma_start(tile[:p, :], x[r0:r0 + p, :])
                ch3 = tile[:p, :].rearrange("p (c w) -> p c w", w=CW)
                if t == NT - 1:
                    # Final tile: per-group colacc updates so each group's PE
                    # transposes start while the DVE still works (the
                    # scheduler runs the reduce first since the updates also
                    # wait on the t17 colacc chain).
                    nc.vector.reduce_max(
                        out=chunkall[:p, t * NC_:(t + 1) * NC_],
                        in_=ch3,
                        axis=mybir.AxisListType.X,
                    )
                    # chunkall is complete here; ship it for the host-side
                    # rowmax / winning-chunk decode (replaces the old
                    # on-device A-encoding tail math, ~2.5 us of DVE).
                    nc.scalar.dma_start(ch_out[:, :], chunkall[:, :])
                    for g in range(4):
                        lo, hi = GB[g], GB[g + 1]
                        nc.vector.tensor_max(
                            colacc[:p, lo:hi], colacc[:p, lo:hi], tile[:p, lo:hi]
                        )
                else:
                    nc.vector.reduce_max(
                        out=chunkall[:p, t * NC_:(t + 1) * NC_],
                        in_=ch3,
                        axis=mybir.AxisListType.X,
                    )
                    nc.vector.tensor_max(colacc[:p, :], colacc[:p, :], tile[:p, :])

            # colacc [128, S] -> colmax via PE transpose chunks + DVE reduce.
            # Groups match GB so group g's transposes only depend on the last
            # tile's group-g update; two PSUM bufs pipeline transpose/reduce.
            for g in range(4):
                c0 = GB[g] // P
                nchunks = (GB[g + 1] - GB[g] + P - 1) // P  # 10,10,10,8
                ps = ppool.tile([P, 10 * P], f32, tag="ps")
                for k in range(nchunks):
                    c = c0 + k
                    # The last chunk (37) is only 64 cols; read the final 128
                    # cols instead (overlapping chunk 36) so the transpose
                    # and the fused group reduce stay full-width. The host
                    # reads chunk 37's values from cm_sb[64:, 37].
                    lo = min(c * P, S - P)
                    nc.tensor.transpose(
                        ps[:, k * P:(k + 1) * P], colacc[:, lo:lo + P],
                        ident[:, :],
                    )
                nc.vector.reduce_max(
                    out=cm_sb[:, c0:c0 + nchunks],
                    in_=ps[:, :nchunks * P].rearrange("p (k q) -> p k q", q=P),
                    axis=mybir.AxisListType.X,
                )
            nc.scalar.dma_start(cmx[:, :], cm_sb[:, :])

    _legalize_waits(nc)
    _BUILT = (nc,)
    return _BUILT


def _border_valid(h, w, b):
    r = np.arange(h)
    c = np.arange(w)
    vr = (r >= b) & (r < h - b)
    vc = (c >= b) & (c < w - b)
    return (vr[:, None] & vc[None, :]).reshape(-1)


def _install_ntff_hook():
    """The image's antenv lacks axon_hooks; recreate it (same ctypes shim the
    boot script would register) so trace=True NTFF profiling works."""
    import types
    import ctypes
    import contextlib

    if "antenv.axon_hooks" in sys.modules:
        return
    so_path = "/opt/axon/libaxon_pjrt.so"
    holder = [None]
    mod = types.ModuleType("antenv.axon_hooks")
    mod.set_axon_ntff_profile_hook = lambda h: holder.__setitem__(0, h)
    mod.get_axon_ntff_profile_hook = lambda: holder[0]
    sys.modules["antenv.axon_hooks"] = mod

    try:
        lib = ctypes.CDLL(so_path)
    except OSError:
        return
    if not hasattr(lib, "axon_start_nrt_profile"):
        return
    lib.axon_start_nrt_profile.argtypes = [
        ctypes.POINTER(ctypes.c_int64),
        ctypes.c_size_t,
    ]
    lib.axon_start_nrt_profile.restype = ctypes.c_int64
    lib.axon_stop_nrt_profile.argtypes = [ctypes.c_char_p]
    lib.axon_stop_nrt_profile.restype = ctypes.c_int64

    @contextlib.contextmanager
    def _hook(output_dir, device_ids):
        import jax

        jax.devices()
        if device_ids:
            ids = (ctypes.c_int64 * len(device_ids))(*device_ids)
            rc = lib.axon_start_nrt_profile(ids, len(device_ids))
        else:
            rc = lib.axon_start_nrt_profile(None, 0)
        if rc != 0:
            raise RuntimeError(f"axon_start_nrt_profile rc={rc}")
        try:
            yield
        finally:
            n = lib.axon_stop_nrt_profile(str(output_dir).encode())
            print(f"profile: {n} file(s) written to {output_dir}", file=sys.stderr)

    holder[0] = _hook


def _run_device(conf, trace=False, trace_kwargs=None):
    (nc,) = _build()
    in_maps = []
    for core in range(8):
        n, half = core // 2, core % 2
        slab = np.ascontiguousarray(conf[n, half * R:(half + 1) * R, :])
        in_maps.append({"x": slab})
    kw = {}
    if trace:
        _install_ntff_hook()
        kw["trace"] = True
        if trace_kwargs:
            kw.update(trace_kwargs)
    res = run_bass_kernel_spmd(nc, in_maps, list(range(8)), **kw)
    return res


def _finalize(conf, results, h0c, w0c, h1c, w1c):
    valid0 = _border_valid(h0c, w0c, BORDER_RM)  # [L]
    valid1 = _border_valid(h1c, w1c, BORDER_RM)  # [S]

    mconf = np.zeros((N, L), np.float32)
    mask_v = np.zeros((N, L), bool)
    all_j = np.zeros((N, L), np.int32)

    def _colpart(cm):
        # chunks 0..36 cover cols 0:4736; the device's chunk-37 transpose
        # reads the last 128 cols (4672:4800), so cols 4736:4800 live in
        # cm[64:, 37].
        return np.concatenate([cm[:, :37].T.ravel(), cm[64:, 37]])

    for n in range(N):
        cm0 = _colpart(results[2 * n]["cmx"])
        cm1 = _colpart(results[2 * n + 1]["cmx"])
        colmax = np.maximum(cm0, cm1)  # [S]
        col_adj = np.where(valid1 & (colmax > THR), colmax, np.inf).astype(np.float32)

        for half in range(2):
            r = results[2 * n + half]
            # ch_out is [P, NT*NC_]: row l = t*P + p holds its NC_ chunk
            # maxima at ch_out[p, t*NC_:(t+1)*NC_].
            chm = r["ch_out"].reshape(P, NT, NC_).transpose(1, 0, 2)
            chm = chm.reshape(NT * P, NC_)[:R]          # [R, NC_]
            rowmax = chm.max(axis=1)                    # [R] f32
            cstar = chm.argmax(axis=1)                  # first winning chunk
            k = (chm == rowmax[:, None]).sum(axis=1)    # winning-chunk count
            rows = slice(half * R, (half + 1) * R)

            # windows of the raw input at each row's winning chunk
            base = cstar * CW                            # [R]
            conf_half = conf[n, rows, :]                 # [R, S] view
            win = np.take_along_axis(
                conf_half, base[:, None] + np.arange(CW)[None, :], axis=1
            )                                            # [R, CW]
            cols = base[:, None] + np.arange(CW)[None, :]
            ok = (
                (win == rowmax[:, None])
                & valid0[rows][:, None]
                & (win == col_adj[cols])
            )
            found = ok.any(axis=1)
            first = np.argmax(ok, axis=1)
            j = np.where(found, base + first, 0).astype(np.int32)
            mc = np.where(found, rowmax, np.float32(0.0)).astype(np.float32)

            mconf[n, rows] = mc
            mask_v[n, rows] = found
            all_j[n, rows] = j

            # Rows whose row max ties across multiple chunks (k != 1): the
            # single-window decode is ambiguous, so recompute them exactly
            # from the raw data (a handful of rows at most).
            suspects = np.nonzero(k != 1)[0]
            for lr in suspects:
                l = half * R + lr
                row = conf[n, l, :]
                m = (
                    (row > THR)
                    & valid0[l]
                    & valid1
                    & (row == row.max())
                    & (row == colmax)
                )
                fv = bool(m.any())
                jj = int(np.argmax(m)) if fv else 0
                mask_v[n, l] = fv
                all_j[n, l] = jj
                mconf[n, l] = row[jj] * np.float32(fv)

    return mconf, mask_v, all_j


def kernel(conf_matrix, h0c, w0c, h1c, w1c):
    conf = np.asarray(conf_matrix, dtype=np.float32)
    assert conf.shape == (N, L, S), conf.shape
    res = _run_device(conf)
    return _finalize(conf, res.results, int(h0c), int(w0c), int(h1c), int(w1c))


def kernel_traced(conf_matrix, h0c, w0c, h1c, w1c, trace_kwargs=None):
    """Like kernel() but with NTFF tracing; returns (outputs, BassKernelResults)."""
    conf = np.asarray(conf_matrix, dtype=np.float32)
    res = _run_device(conf, trace=True, trace_kwargs=trace_kwargs)
    out = _finalize(conf, res.results, int(h0c), int(w0c), int(h1c), int(w1c))
    return out, res



# revision 48
# speedup vs baseline: 1.1930x; 1.1930x over previous
"""Trainium2 Bass kernel for CoarseMatching (mutual-nearest-neighbor + border/thr masking).

Contract: kernel(**inputs) takes the FULL inputs (conf_matrix [4,4800,4800] f32 plus
scalar grid dims) and returns the FULL outputs (mconf [4,4800] f32, mask_v [4,4800] bool,
all_j_ids [4,4800] int32), matching reference() exactly.

Strategy (8 NeuronCores, single kernel launch, ~217 us HW):
  - Shard each of the 4 samples' rows across 2 cores -> per-core slab [2400, 4800].
  - One streaming pass per core over 19 row-tiles [128, 4800]; per tile the DVE does
    exactly two full passes over the data (the floor for exact mutual-NN: every
    combining op - tensor_reduce, fp32 tensor_tensor, tensor_scalar+accum - runs at
    1 elem/cycle/lane on TRN2, and no other engine can legally run 2-tensor max in
    this toolchain):
      1. chunked row maxima: reduce_max over [128, 25, 192] -> chunk maxima
      2. running column-max accumulator (tensor_max)
    Tile 0 streams in as five 5-chunk pieces (and tile 1 as two halves) so the DVE
    starts ~2 us after the fixed preamble and is fed gaplessly; tile 0's colacc
    contribution uses 2x-mode tensor_copy. The last tile's colacc updates are
    split into four 128-aligned column groups so the PE-transpose epilogue starts
    while the DVE still drains.
  - Tail (tiny): rowmax per row from chunk maxima; an integer-encoded weighted sum
    A = sum_c [chunkmax>=rowmax]*(65536 + 25 - c) identifies each row's winning chunk
    (and flags multi-chunk row-max ties via the 65536 multiplicity). PE-transpose +
    DVE-reduce epilogue turns the column accumulator into per-core partial colmax.
  - Host: combine partial colmaxes per sample pair; decode the winning chunk; re-read
    only the 192-wide window of the raw input per row to find the first mask index
    (conf==rowmax AND conf==colmax AND border/thr). Rows with multi-chunk ties (~5 per
    run) are recomputed exactly from the raw row. Bitwise-exact vs the reference.
"""

import sys

if "/opt/trn_rl_repo" not in sys.path:
    sys.path.insert(0, "/opt/trn_rl_repo")

import numpy as np

import concourse.bass as bass
import concourse.mybir as mybir
from concourse.masks import make_identity
from concourse.tile import TileContext
from concourse.vector_clock import ScopedClock, VectorClock
from concourse.bass_utils import run_bass_kernel_spmd

THR = 0.2
BORDER_RM = 2

N = 4
L = 4800
S = 4800
R = L // 2          # rows per core
P = 128
NFULL = R // P      # 18 full tiles
TAIL = R - NFULL * P  # 96
NT = NFULL + 1
NCHUNK = (S + P - 1) // P  # 38 column chunks for colmax transpose-reduce

CW = 192            # row-chunk width for rowmax/argmax chunking
NC_ = S // CW       # 25 chunks per row
WBASE = 65536.0     # chunk-id encoding base (exact in f32 up to 2^24 sums)

_BUILT = None  # cached (nc,) bass program


def _patched_drain_and_barrier(self, tick_clock, wait_clock):
    # The stock tile-exit drain carries one sem-wait per live semaphore; this
    # walrus build only encodes 1 sync wait per CTRL instruction. Split the
    # waits across single-wait SP NOPs, then drain with none attached.
    gc = tick_clock.global_clock
    vc = gc[None] if hasattr(gc, "items") else gc
    n = len(vc)
    for p in range(n):
        if vc[p] > 0:
            sub = [0] * n
            sub[p] = vc[p]
            nop_inst = self.nc.sync.nop()
            wait_clock.add_sem_waits(nop_inst.ins, ScopedClock({None: VectorClock(sub)}))
    self.nc.sync.drain()
    self.nc.all_engine_barrier()
    assert self.sems is not None
    popped = self.nc._tile_sem_poison_stack.pop()
    assert popped is self._sem_poison
    self.nc.clear_and_free_semaphores(list(self.sems.allocated().values()))
    self.nc.all_engine_barrier()


def _legalize_waits(nc):
    """This walrus build encodes at most ONE sync wait per instruction; Tile's
    scheduler attaches up to 4. Split the extras onto same-engine NOPs placed
    immediately before the instruction (same program order, same semantics)."""
    ctr = [0]

    def mknop(engine, wait):
        ctr[0] += 1
        return mybir.InstNoOp(
            name=f"I-wsplit-{ctr[0]}",
            engine=engine,
            ins=[],
            outs=[],
            sync_info=mybir.SyncInfo(on_wait=[wait], on_update=[]),
        )

    f = nc.m.functions[0]
    for bb in f.blocks:
        insts = list(bb.instructions)
        out = []
        changed = False
        for inst in insts:
            si = inst.sync_info
            waits = list(si.on_wait) if si is not None else []
            if len(waits) > 1:
                ups = list(si.on_update) if si is not None else []
                for w in waits[:-1]:
                    out.append(mknop(inst.engine, w))
                inst.sync_info = mybir.SyncInfo(on_wait=[waits[-1]], on_update=ups)
                changed = True
            out.append(inst)
        if changed:
            bb.instructions = out


def _build():
    global _BUILT
    if _BUILT is not None:
        return _BUILT

    TileContext._drain_and_barrier = _patched_drain_and_barrier

    nc = bass.Bass("TRN2")
    f32 = mybir.dt.float32

    x = nc.dram_tensor("x", [R, S], f32, kind="ExternalInput")
    ch_out = nc.dram_tensor("ch_out", [P, NT * NC_], f32, kind="ExternalOutput")
    cmx = nc.dram_tensor("cmx", [P, NCHUNK], f32, kind="ExternalOutput")

    with TileContext(nc) as tc:
        with (
            tc.tile_pool(name="data", bufs=6) as dpool,
            tc.tile_pool(name="acc", bufs=1) as apool,
            tc.tile_pool(name="psum", bufs=2, space="PSUM") as ppool,
        ):
            colacc = apool.tile([P, S], f32)
            chunkall = apool.tile([P, NT * NC_], f32)
            cm_sb = apool.tile([P, NCHUNK], f32)
            ident = apool.tile([P, P], f32)

            # Column groups for the last tile's colacc update + epilogue
            # transposes: 128-aligned so each group's PE transposes can start
            # as soon as that group's final update lands.
            GB = [0, 1280, 2560, 3840, S]  # chunks 0-9, 10-19, 20-29, 30-37

            # Tile 0 first as five 5-chunk pieces so the DVE starts ~2 us
            # after the preamble and is fed continuously (piece transfer
            # ~1.5 us vs ~1.6 us of DVE work per piece); its colacc
            # contribution is a 2x-mode tensor_copy per piece. Tile 1 in two
            # halves to bridge into the steady stream; tiles 2..17 whole;
            # tile 18 (96 rows) last with per-group colacc updates feeding
            # the PE-transpose epilogue.
            NP0 = 5
            for k in range(NP0):
                lo, hi = k * 5 * CW, (k + 1) * 5 * CW
                tp = apool.tile([P, hi - lo], f32, tag=f"t0_{k}")
                nc.sync.dma_start(tp[:, :], x[:P, lo:hi])
                nc.vector.reduce_max(
                    out=chunkall[:, lo // CW:hi // CW],
                    in_=tp[:, :].rearrange("p (c w) -> p c w", w=CW),
                    axis=mybir.AxisListType.X,
                )
                nc.vector.tensor_copy(colacc[:, lo:hi], tp[:, :])
                if k == 0:
                    # Epilogue-only setup, placed in the DMA-wait gaps.
                    make_identity(nc, ident[:, :])
                    nc.vector.memset(chunkall[TAIL:, NFULL * NC_:], 0.0)
                    w37 = S - 37 * P
                    nc.vector.memset(cm_sb[w37:, 37:38], 0.0)

            # Tile 1 in two halves (12|13 chunks).
            mid1 = 12 * CW
            for lo, hi in ((0, mid1), (mid1, S)):
                th = apool.tile([P, hi - lo], f32, tag=f"t1_{lo}")
                nc.sync.dma_start(th[:, :], x[P:2 * P, lo:hi])
                nc.vector.reduce_max(
                    out=chunkall[:, NC_ + lo // CW:NC_ + hi // CW],
                    in_=th[:, :].rearrange("p (c w) -> p c w", w=CW),
                    axis=mybir.AxisListType.X,
                )
                nc.vector.tensor_max(
                    colacc[:, lo:hi], colacc[:, lo:hi], th[:, :]
                )

            for t in range(2, NT):
                p = P if t < NFULL else TAIL
                r0 = t * P
                tile = dpool.tile([P, S], f32, tag="tile")
                nc.sync.dma_start(tile[:p, :], x[r0:r0 + p, :])
                ch3 = tile[:p, :].rearrange("p (c w) -> p c w", w=CW)
                if t == NT - 1:
                    # Final tile: per-group colacc updates so each group's PE
                    # transposes start while the DVE still works (the
                    # scheduler runs the reduce first since the updates also
                    # wait on the t17 colacc chain).
                    nc.vector.reduce_max(
                        out=chunkall[:p, t * NC_:(t + 1) * NC_],
                        in_=ch3,
                        axis=mybir.AxisListType.X,
                    )
                    # chunkall is complete here; ship it for the host-side
                    # rowmax / winning-chunk decode (replaces the old
                    # on-device A-encoding tail math, ~2.5 us of DVE).
                    nc.scalar.dma_start(ch_out[:, :], chunkall[:, :])
                    for g in range(4):
                        lo, hi = GB[g], GB[g + 1]
                        nc.vector.tensor_max(
                            colacc[:p, lo:hi], colacc[:p, lo:hi], tile[:p, lo:hi]
                        )
                else:
                    nc.vector.reduce_max(
                        out=chunkall[:p, t * NC_:(t + 1) * NC_],
                        in_=ch3,
                        axis=mybir.AxisListType.X,
                    )
                    nc.vector.tensor_max(colacc[:p, :], colacc[:p, :], tile[:p, :])

            # colacc [128, S] -> colmax via PE transpose chunks + DVE reduce.
            # Groups match GB so group g's transposes only depend on the last
            # tile's group-g update; two PSUM bufs pipeline transpose/reduce.
            for g in range(4):
                c0 = GB[g] // P
                nchunks = (GB[g + 1] - GB[g] + P - 1) // P  # 10,10,10,8
                ps = ppool.tile([P, 10 * P], f32, tag="ps")
                for k in range(nchunks):
                    c = c0 + k
                    w = min(P, S - c * P)
                    nc.tensor.transpose(
                        ps[:w, k * P:(k + 1) * P], colacc[:, c * P:c * P + w],
                        ident[:, :],
                    )
                nfull = nchunks if g < 3 else nchunks - 1
                nc.vector.reduce_max(
                    out=cm_sb[:, c0:c0 + nfull],
                    in_=ps[:, :nfull * P].rearrange("p (k q) -> p k q", q=P),
                    axis=mybir.AxisListType.X,
                )
                if g == 3:  # ragged chunk 37 (64 cols)
                    w37 = S - 37 * P
                    nc.vector.reduce_max(
                        out=cm_sb[:w37, 37:38],
                        in_=ps[:w37, (nchunks - 1) * P:(nchunks - 1) * P + P],
                        axis=mybir.AxisListType.X,
                    )
            nc.scalar.dma_start(cmx[:, :], cm_sb[:, :])

    _legalize_waits(nc)
    _BUILT = (nc,)
    return _BUILT


def _border_valid(h, w, b):
    r = np.arange(h)
    c = np.arange(w)
    vr = (r >= b) & (r < h - b)
    vc = (c >= b) & (c < w - b)
    return (vr[:, None] & vc[None, :]).reshape(-1)


def _install_ntff_hook():
    """The image's antenv lacks axon_hooks; recreate it (same ctypes shim the
    boot script would register) so trace=True NTFF profiling works."""
    import types
    import ctypes
    import contextlib

    if "antenv.axon_hooks" in sys.modules:
        return
    so_path = "/opt/axon/libaxon_pjrt.so"
    holder = [None]
    mod = types.ModuleType("antenv.axon_hooks")
    mod.set_axon_ntff_profile_hook = lambda h: holder.__setitem__(0, h)
    mod.get_axon_ntff_profile_hook = lambda: holder[0]
    sys.modules["antenv.axon_hooks"] = mod

    try:
        lib = ctypes.CDLL(so_path)
    except OSError:
        return
    if not hasattr(lib, "axon_start_nrt_profile"):
        return
    lib.axon_start_nrt_profile.argtypes = [
        ctypes.POINTER(ctypes.c_int64),
        ctypes.c_size_t,
    ]
    lib.axon_start_nrt_profile.restype = ctypes.c_int64
    lib.axon_stop_nrt_profile.argtypes = [ctypes.c_char_p]
    lib.axon_stop_nrt_profile.restype = ctypes.c_int64

    @contextlib.contextmanager
    def _hook(output_dir, device_ids):
        import jax

        jax.devices()
        if device_ids:
            ids = (ctypes.c_int64 * len(device_ids))(*device_ids)
            rc = lib.axon_start_nrt_profile(ids, len(device_ids))
        else:
            rc = lib.axon_start_nrt_profile(None, 0)
        if rc != 0:
            raise RuntimeError(f"axon_start_nrt_profile rc={rc}")
        try:
            yield
        finally:
            n = lib.axon_stop_nrt_profile(str(output_dir).encode())
            print(f"profile: {n} file(s) written to {output_dir}", file=sys.stderr)

    holder[0] = _hook


def _run_device(conf, trace=False, trace_kwargs=None):
    (nc,) = _build()
    in_maps = []
    for core in range(8):
        n, half = core // 2, core % 2
        slab = np.ascontiguousarray(conf[n, half * R:(half + 1) * R, :])
        in_maps.append({"x": slab})
    kw = {}
    if trace:
        _install_ntff_hook()
        kw["trace"] = True
        if trace_kwargs:
            kw.update(trace_kwargs)
    res = run_bass_kernel_spmd(nc, in_maps, list(range(8)), **kw)
    return res


def _finalize(conf, results, h0c, w0c, h1c, w1c):
    valid0 = _border_valid(h0c, w0c, BORDER_RM)  # [L]
    valid1 = _border_valid(h1c, w1c, BORDER_RM)  # [S]

    mconf = np.zeros((N, L), np.float32)
    mask_v = np.zeros((N, L), bool)
    all_j = np.zeros((N, L), np.int32)

    for n in range(N):
        cm0 = results[2 * n]["cmx"].T.ravel()[:S]
        cm1 = results[2 * n + 1]["cmx"].T.ravel()[:S]
        colmax = np.maximum(cm0, cm1)  # [S]
        col_adj = np.where(valid1 & (colmax > THR), colmax, np.inf).astype(np.float32)

        for half in range(2):
            r = results[2 * n + half]
            # ch_out is [P, NT*NC_]: row l = t*P + p holds its NC_ chunk
            # maxima at ch_out[p, t*NC_:(t+1)*NC_].
            chm = r["ch_out"].reshape(P, NT, NC_).transpose(1, 0, 2)
            chm = chm.reshape(NT * P, NC_)[:R]          # [R, NC_]
            rowmax = chm.max(axis=1)                    # [R] f32
            cstar = chm.argmax(axis=1)                  # first winning chunk
            k = (chm == rowmax[:, None]).sum(axis=1)    # winning-chunk count
            rows = slice(half * R, (half + 1) * R)

            # windows of the raw input at each row's winning chunk
            base = cstar * CW                            # [R]
            conf_half = conf[n, rows, :]                 # [R, S] view
            win = np.take_along_axis(
                conf_half, base[:, None] + np.arange(CW)[None, :], axis=1
            )                                            # [R, CW]
            cols = base[:, None] + np.arange(CW)[None, :]
            ok = (
                (win == rowmax[:, None])
                & valid0[rows][:, None]
                & (win == col_adj[cols])
            )
            found = ok.any(axis=1)
            first = np.argmax(ok, axis=1)
            j = np.where(found, base + first, 0).astype(np.int32)
            mc = np.where(found, rowmax, np.float32(0.0)).astype(np.float32)

            mconf[n, rows] = mc
            mask_v[n, rows] = found
            all_j[n, rows] = j

            # Rows whose row max ties across multiple chunks (k != 1): the
            # single-window decode is ambiguous, so recompute them exactly
            # from the raw data (a handful of rows at most).
            suspects = np.nonzero(k != 1)[0]
            for lr in suspects:
                l = half * R + lr
                row = conf[n, l, :]
                m = (
                    (row > THR)
                    & valid0[l]
                    & valid1
                    & (row == row.max())
                    & (row == colmax)
                )
                fv = bool(m.any())
                jj = int(np.argmax(m)) if fv else 0
                mask_v[n, l] = fv
                all_j[n, l] = jj
                mconf[n, l] = row[jj] * np.float32(fv)

    return mconf, mask_v, all_j


def kernel(conf_matrix, h0c, w0c, h1c, w1c):
    conf = np.asarray(conf_matrix, dtype=np.float32)
    assert conf.shape == (N, L, S), conf.shape
    res = _run_device(conf)
    return _finalize(conf, res.results, int(h0c), int(w0c), int(h1c), int(w1c))


def kernel_traced(conf_matrix, h0c, w0c, h1c, w1c, trace_kwargs=None):
    """Like kernel() but with NTFF tracing; returns (outputs, BassKernelResults)."""
    conf = np.asarray(conf_matrix, dtype=np.float32)
    res = _run_device(conf, trace=True, trace_kwargs=trace_kwargs)
    out = _finalize(conf, res.results, int(h0c), int(w0c), int(h1c), int(w1c))
    return out, res

